# revision 4
# baseline (speedup 1.0000x reference)
"""Trainium2 Bass kernel for DeepMultiOmicPathwayNet.

Model (per batch row n):
  g    = x[n, pathway_ids, :]                  -> [P=200, K*C=192]
  t    = einsum('pi,pio->po', g, W_path) + b_path      (per-pathway linear)
  t    = t / ||t||_2 (row L2 over each pathway's 64 outputs)
  ncb  = x[n, nc_ids, :].flatten() @ W_nc + b_nc       ([15000] @ [15000,512])
  h    = sigmoid(concat(t.flatten(), ncb))             ([13312])
  out  = h @ W_out + b_out                             ([20])

Strategy: data-parallel over batch N=1024 across 8 cores (128 rows/core).
Host prep does the (compile-time-known) gathers + transposes + bf16 cast and
folds every bias into the matmuls by appending a ones-row to the data and the
bias as an extra contraction row of the weights. On device, per core:

  phase A: 200 pathway matmuls ([193,128]^T @ [193,64] in 2 K-chunks) -> PSUM,
           copy to bf16 SBUF, per-pathway sum-of-squares on DVE.
  phase B: one batched sqrt (ACT) + reciprocal (DVE)  -> 1/||t||  (single
           activation-table load; avoids 200 table switches).
  phase C: 118-tile K-accumulated matmul for the non-cancer branch,
           interleaved (for PE/DMA overlap) with per-pathway-pair
           sigmoid (scale=1/||t||) -> PE transpose -> feature-major bf16
           chunk -> accumulate chunk @ W_out tile into the [128,20] output
           PSUM accumulator.
  tail:    nc-branch sigmoid + 4 transposes + final W_out tiles + b_out
           (ones-outer-product matmul), copy PSUM -> SBUF, DMA out.

Everything streams in bf16 (PSUM accumulation in f32).
"""
import numpy as np
import ml_dtypes

import concourse.bass as bass
import concourse.bacc as bacc
import concourse.tile as tile
import concourse.mybir as mybir
from concourse.bass_utils import run_bass_kernel_spmd
from concourse.masks import make_identity

bf16 = mybir.dt.bfloat16
f32 = mybir.dt.float32
BF = ml_dtypes.bfloat16

N, G, C = 1024, 20000, 3
P, K = 200, 64
KC = K * C              # 192
NCG = 5000              # non-cancer genes
HID = 512
OUT = 20
NB = 128                # batch rows per core
NCORES = 8
NPAIR = P // 2          # 100
NKT = 118               # K tiles for nc branch: 118*128 = 15104 >= 15001
NKROWS = NKT * 128
NFT = P * K // 128      # 100 feature tiles from pathways
NFT_NC = HID // 128     # 4 feature tiles from nc branch
FEAT = P * K + HID      # 13312

_CACHE = {}


def _build(npair=NPAIR, nkt=NKT, nft_nc=NFT_NC, tail=True):
    """npair/nkt/nft_nc scale down the loops for bisection; full kernel by default."""
    nc = bacc.Bacc(None, target_bir_lowering=False)

    pd_hi_d = nc.declare_dram_parameter("pd_hi", [NPAIR, 128, 256], bf16, isOutput=False)
    pd_lo_d = nc.declare_dram_parameter("pd_lo", [NPAIR, 65, 256], bf16, isOutput=False)
    wphi_d = nc.declare_dram_parameter("wphi", [128, P, K], bf16, isOutput=False)
    wplo_d = nc.declare_dram_parameter("wplo", [65, P, K], bf16, isOutput=False)
    ncd_d = nc.declare_dram_parameter("ncd", [NKROWS, NB], bf16, isOutput=False)
    wnc_d = nc.declare_dram_parameter("wnc", [NKROWS, HID], bf16, isOutput=False)
    wout_d = nc.declare_dram_parameter("wout", [128, NFT + NFT_NC, OUT], bf16, isOutput=False)
    bout_d = nc.declare_dram_parameter("bout", [1, OUT], bf16, isOutput=False)
    out_d = nc.declare_dram_parameter("out", [NB, OUT], f32, isOutput=True)

    with tile.TileContext(nc) as tc:
        with (
            tc.tile_pool(name="cst", bufs=1) as cst,
            tc.tile_pool(name="pd", bufs=3) as pd,
            tc.tile_pool(name="ncw", bufs=4) as ncw,
            tc.tile_pool(name="sig", bufs=3) as sig,
            tc.tile_pool(name="tp", bufs=2, space="PSUM") as tp,
            tc.tile_pool(name="stp", bufs=2, space="PSUM") as stp,
            tc.tile_pool(name="ncp", bufs=1, space="PSUM") as ncp,
            tc.tile_pool(name="outp", bufs=1, space="PSUM") as outp,
        ):
            ident = cst.tile([128, 128], bf16)
            make_identity(nc, ident[:])
            ones_t = cst.tile([1, 128], bf16)
            nc.gpsimd.memset(ones_t[:], 1.0)

            wphi_sb = cst.tile([128, P, K], bf16)
            nc.sync.dma_start(wphi_sb[:], wphi_d[:])
            wplo_sb = cst.tile([65, P, K], bf16)
            nc.sync.dma_start(wplo_sb[:], wplo_d[:])
            wout_sb = cst.tile([128, NFT + NFT_NC, OUT], bf16)
            nc.sync.dma_start(wout_sb[:], wout_d[:])
            bout_sb = cst.tile([1, OUT], bf16)
            nc.sync.dma_start(bout_sb[:], bout_d[:])

            t_all = cst.tile([NB, P, K], bf16)
            ss_all = cst.tile([NB, P], f32)
            inv_all = cst.tile([NB, P], f32)

            # ---------- phase A: pathway matmuls + sumsq ----------
            for j in range(npair):
                pdh = pd.tile([128, 256], bf16)
                nc.sync.dma_start(pdh[:], pd_hi_d[j])
                pdl = pd.tile([65, 256], bf16)
                nc.sync.dma_start(pdl[:], pd_lo_d[j])
                t_ps = tp.tile([NB, 2, K], f32)
                for jj in range(2):
                    p = 2 * j + jj
                    nc.tensor.matmul(t_ps[:, jj, :], pdh[:, jj * 128:(jj + 1) * 128],
                                     wphi_sb[:, p, :], start=True, stop=False)
                    nc.tensor.matmul(t_ps[:, jj, :], pdl[:, jj * 128:(jj + 1) * 128],
                                     wplo_sb[:, p, :], start=False, stop=True)
                nc.vector.tensor_copy(t_all[:, 2 * j:2 * j + 2, :], t_ps[:])
                sq = pd.tile([NB, K], bf16)
                for jj in range(2):
                    p = 2 * j + jj
                    nc.scalar.activation(
                        sq[:], t_ps[:, jj, :],
                        mybir.ActivationFunctionType.Square,
                        accum_out=ss_all[:, p:p + 1],
                    )

            # ---------- phase B: 1/norm, batched (one table load) ----------
            nc.scalar.sqrt(inv_all[:], ss_all[:])
            nc.vector.reciprocal(inv_all[:], inv_all[:])

            # ---------- phase C: nc-branch matmul interleaved with pathway finalize ----
            nc_ps = ncp.tile([NB, HID], f32)
            out_ps = outp.tile([NB, OUT], f32)
            for step in range(nkt):
                ncd_t = ncw.tile([128, NB], bf16)
                nc.sync.dma_start(ncd_t[:], ncd_d[step * 128:(step + 1) * 128, :])
                wnc_t = ncw.tile([128, HID], bf16)
                nc.sync.dma_start(wnc_t[:], wnc_d[step * 128:(step + 1) * 128, :])

                if step < npair:
                    j = step
                    s_pair = sig.tile([NB, 128], bf16)
                    for jj in range(2):
                        p = 2 * j + jj
                        nc.scalar.activation(
                            s_pair[:, jj * K:(jj + 1) * K], t_all[:, p, :],
                            mybir.ActivationFunctionType.Sigmoid,
                            scale=inv_all[:, p:p + 1],
                        )
                    st_ps = stp.tile([128, NB], bf16)
                    nc.tensor.transpose(st_ps[:], s_pair[:], ident[:])
                    hT = sig.tile([128, NB], bf16)
                    nc.vector.tensor_copy(hT[:], st_ps[:])
                    nc.tensor.matmul(out_ps[:], hT[:], wout_sb[:, j, :],
                                     start=(j == 0), stop=False)

                nc.tensor.matmul(nc_ps[:], ncd_t[:], wnc_t[:],
                                 start=(step == 0), stop=(step == nkt - 1))

            # ---------- tail: nc sigmoid, transposes, final tiles, bias, out ----------
            s_nc = cst.tile([NB, HID], bf16)
            nc.scalar.activation(s_nc[:], nc_ps[:],
                                 mybir.ActivationFunctionType.Sigmoid)
            for i in range(nft_nc):
                st_ps = stp.tile([128, NB], bf16)
                nc.tensor.transpose(st_ps[:], s_nc[:, i * 128:(i + 1) * 128], ident[:])
                hT = sig.tile([128, NB], bf16)
                nc.vector.tensor_copy(hT[:], st_ps[:])
                nc.tensor.matmul(out_ps[:], hT[:], wout_sb[:, NFT + i, :],
                                 start=False, stop=False)
            nc.tensor.matmul(out_ps[:], ones_t[:], bout_sb[:],
                             start=False, stop=True)

            out_sb = cst.tile([NB, OUT], f32)
            nc.vector.tensor_copy(out_sb[:], out_ps[:])
            nc.sync.dma_start(out_d[:], out_sb[:])

    nc.compile()
    return nc


def _prep(inputs):
    x = np.asarray(inputs["x"], np.float32)
    pathway_ids = np.asarray(inputs["pathway_ids"]).astype(np.int64)
    nc_ids = np.asarray(inputs["nc_ids"]).astype(np.int64)
    W_path = np.asarray(inputs["W_path"], np.float32)
    b_path = np.asarray(inputs["b_path"], np.float32)
    W_nc = np.asarray(inputs["W_nc"], np.float32)
    b_nc = np.asarray(inputs["b_nc"], np.float32)
    W_out = np.asarray(inputs["W_out"], np.float32)
    b_out = np.asarray(inputs["b_out"], np.float32)

    n = x.shape[0]
    xt = np.ascontiguousarray(x.reshape(n, G * C).T)            # [60000, n]

    pidx = ((pathway_ids * 3)[:, :, None] + np.arange(3)).reshape(-1)
    prows = xt[pidx].reshape(P, KC, n)                          # [200, 192, n]
    ph = prows[:, 0:128, :]
    pl = np.concatenate([prows[:, 128:KC, :], np.ones((P, 1, n), np.float32)], axis=1)
    ph_pair = np.ascontiguousarray(ph.reshape(NPAIR, 2, 128, n).transpose(0, 2, 1, 3)).astype(BF)
    pl_pair = np.ascontiguousarray(pl.reshape(NPAIR, 2, 65, n).transpose(0, 2, 1, 3)).astype(BF)

    nidx = ((nc_ids * 3)[:, None] + np.arange(3)).reshape(-1)
    ncd_all = np.zeros((NKROWS, n), np.float32)
    ncd_all[:NCG * C] = xt[nidx]
    ncd_all[NCG * C] = 1.0
    ncd_all = ncd_all.astype(BF)

    wphi = np.ascontiguousarray(W_path[:, 0:128, :].transpose(1, 0, 2)).astype(BF)
    wplo = np.ascontiguousarray(
        np.concatenate([W_path[:, 128:KC, :], b_path[:, None, :]], axis=1).transpose(1, 0, 2)
    ).astype(BF)
    wnc_aug = np.zeros((NKROWS, HID), np.float32)
    wnc_aug[:NCG * C] = W_nc
    wnc_aug[NCG * C] = b_nc
    wnc_aug = wnc_aug.astype(BF)
    wout_t = np.ascontiguousarray(
        W_out.reshape(NFT + NFT_NC, 128, OUT).transpose(1, 0, 2)
    ).astype(BF)
    bout = b_out.reshape(1, OUT).astype(BF)

    in_maps = []
    for c in range(NCORES):
        sl = slice(c * NB, (c + 1) * NB)
        in_maps.append({
            "pd_hi": np.ascontiguousarray(ph_pair[:, :, :, sl]).reshape(NPAIR, 128, 256),
            "pd_lo": np.ascontiguousarray(pl_pair[:, :, :, sl]).reshape(NPAIR, 65, 256),
            "wphi": wphi,
            "wplo": wplo,
            "ncd": np.ascontiguousarray(ncd_all[:, sl]),
            "wnc": wnc_aug,
            "wout": wout_t,
            "bout": bout,
        })
    return in_maps


def kernel(**inputs):
    if "nc" not in _CACHE:
        _CACHE["nc"] = _build()
    nc = _CACHE["nc"]
    in_maps = _prep(inputs)
    res = run_bass_kernel_spmd(nc, in_maps, list(range(NCORES)), **_CACHE.get("run_kwargs", {}))
    _CACHE["last_result"] = res
    return np.concatenate([res.results[c]["out"] for c in range(NCORES)], axis=0)


if __name__ == "__main__":
    rng = np.random.default_rng(0)
    print("building only...")
    _build()
    print("build OK")
